# revision 10
# baseline (speedup 1.0000x reference)
"""Causal self-attention with RoPE on 8 Trainium2 NeuronCores.

Sharding: tensor-parallel over heads (16 heads -> 2 per core) for
QKV projections, RoPE and attention; AllToAll re-shards the attention
output from head-sharded to token-sharded; the output projection then
runs token-parallel (each core computes all 2048 output features for
its 512 tokens), so no all-reduce is needed.

Shapes (hardcoded): x [2, 2048, 2048], W_* [2048, 2048], 16 heads,
d_k = 128, fp32 in/out.

On-chip dataflow per core (all matmuls via PE, contraction on the
partition axis):
  - xT chunks [128d x (16kb x 256t)] stream in; per head h:
      qT/kT [128dk, 256t] = sum_kb Wq_h_kb.T @ xT_kb   (PSUM)
      RoPE applied with a stream_shuffle pair-swap + 2 muls + add
  - v in natural [token, d] layout: v = x_blk @ Wv.T
  - attention works on transposed scores: ST[j*128 keys, 512 q] =
      kT_j.T @ qT_i ; p = exp(ST + causal_mask); l += ones.T @ p;
      outT += v_j.T(@natural v) @ p   -- no max-subtraction needed
      (logits are O(1) by construction), no transposes anywhere.
  - normalize: r = 1/l broadcast via a K=1 matmul, y = outT * r
  - AllToAll: y (head-shard) -> yfull slice (token-shard)
  - out projection: outT_e = sum_db WoT_db_e.T @ yT_db  for the core's
    512 tokens.
"""

import sys
import time

for _p in ("/opt/trn_rl_repo", "/opt/pypackages"):
    if _p not in sys.path:
        sys.path.insert(0, _p)

import numpy as np

import concourse.bass as bass
import concourse.bacc as bacc
import concourse.mybir as mybir
import concourse.tile as tile
from concourse import bass_utils
from concourse.alu_op_type import AluOpType

# ---------------------------------------------------------------- config
N_CORES = 8
B, S, D = 2, 2048, 2048
H = 16
DK = D // H              # 128
HPC = H // N_CORES       # 2 heads per core
TOK = B * S              # 4096
SUB = 256                # token sub-chunk for projections
QCH = 512                # attention query chunk
JB = 128                 # attention key block
NSUB = TOK // SUB        # 16
KB = D // 128            # 16 contraction blocks
ROPE_BASE = 10000.0
MASK_NEG = -30000.0

# "f32" (exact, 4x slower matmul), "f32r" (full-rate fp32-storage
# reduced-precision matmul), "bf16"
DT_MODE = "f32r"

F32 = mybir.dt.float32


def _dt_mm():
    if DT_MODE == "bf16":
        return mybir.dt.bfloat16
    if DT_MODE == "f32r":
        return mybir.dt.float32r
    return F32


def _mm_view(ap):
    """Matmul-operand view; tiles are already in the matmul dtype."""
    return ap


def _np_dt():
    if DT_MODE == "bf16":
        import ml_dtypes
        return np.dtype(ml_dtypes.bfloat16)
    return np.dtype(np.float32)


# ---------------------------------------------------------------- build
_CACHE = {}


def _build_nc():
    dt = _dt_mm()
    nc = bacc.Bacc("TRN2", target_bir_lowering=False, debug=False,
                   num_devices=N_CORES)

    xT = nc.dram_tensor("xT", [D, TOK], dt, kind="ExternalInput")
    wqT = nc.dram_tensor("wqT", [D, HPC * DK], dt, kind="ExternalInput")
    wkT = nc.dram_tensor("wkT", [D, HPC * DK], dt, kind="ExternalInput")
    wvT = nc.dram_tensor("wvT", [D, HPC * DK], dt, kind="ExternalInput")
    woT = nc.dram_tensor("woT", [D, D], dt, kind="ExternalInput")
    ropeC = nc.dram_tensor("ropeC", [DK, S], F32, kind="ExternalInput")
    ropeS = nc.dram_tensor("ropeS", [DK, S], F32, kind="ExternalInput")
    maskd = nc.dram_tensor("maskd", [JB, 4 * QCH], F32, kind="ExternalInput")
    outT = nc.dram_tensor("outT", [D, QCH], F32, kind="ExternalOutput")

    swap_mask = [i ^ 1 for i in range(32)]

    import contextlib
    with tile.TileContext(nc) as tc:
        with contextlib.ExitStack() as st_outer:
            dram = st_outer.enter_context(
                tc.tile_pool(name="dram", bufs=1, space="DRAM"))
            # collective buffers (DRAM)
            y_a2a = dram.tile([D, QCH], dt)
            yfull = dram.tile([D, QCH], dt)

            st_main = st_outer.enter_context(contextlib.ExitStack())
            const = st_main.enter_context(tc.tile_pool(name="const", bufs=1))
            xpool = st_main.enter_context(tc.tile_pool(name="xpool", bufs=2))
            qpool = st_main.enter_context(tc.tile_pool(name="qpool", bufs=4))
            kvpool = st_main.enter_context(tc.tile_pool(name="kvpool", bufs=8))
            vpool = st_main.enter_context(tc.tile_pool(name="vpool", bufs=16))
            work = st_main.enter_context(tc.tile_pool(name="work", bufs=2))
            ppool = st_main.enter_context(tc.tile_pool(name="ppool", bufs=3))
            ps_proj = st_main.enter_context(
                tc.tile_pool(name="ps_proj", bufs=2, space="PSUM"))
            ps_st = st_main.enter_context(
                tc.tile_pool(name="ps_st", bufs=2, space="PSUM"))
            ps_out = st_main.enter_context(
                tc.tile_pool(name="ps_out", bufs=2, space="PSUM"))
            ps_misc = st_main.enter_context(
                tc.tile_pool(name="ps_misc", bufs=1, space="PSUM"))

            # ---- persistent constants in SBUF
            wq_sb = const.tile([128, KB * HPC * DK], dt)
            wk_sb = const.tile([128, KB * HPC * DK], dt)
            wv_sb = const.tile([128, KB * HPC * DK], dt)
            for sb_t, dr in ((wq_sb, wqT), (wk_sb, wkT), (wv_sb, wvT)):
                nc.sync.dma_start(
                    sb_t.rearrange("p (kb m) -> p kb m", kb=KB),
                    dr.ap().rearrange("(kb p) m -> p kb m", p=128))
            ropeC_sb = const.tile([DK, S], F32)
            ropeS_sb = const.tile([DK, S], F32)
            maskd_sb = const.tile([JB, 4 * QCH], F32)
            nc.sync.dma_start(ropeC_sb[:], ropeC[:])
            nc.sync.dma_start(ropeS_sb[:], ropeS[:])
            nc.sync.dma_start(maskd_sb[:], maskd[:])
            ones_col_f32 = const.tile([128, 1], F32)
            ones_row = const.tile([1, 128], F32)
            nc.vector.memset(ones_col_f32[:], 1.0)
            nc.vector.memset(ones_row[:], 1.0)
            if dt == F32:
                ones_col = ones_col_f32
            else:
                ones_col = const.tile([128, 1], dt)
                nc.vector.tensor_copy(ones_col[:], ones_col_f32[:])

            qT_tiles = {}
            kT_tiles = {}
            v_tiles = {}

            def rope_combine(ps_in, out_ap, s0, n):
                """out = ps_in * C + shuffle(ps_in) * S  (RoPE)."""
                qsh = work.tile([128, SUB], F32, tag="qsh")
                t1 = work.tile([128, SUB], F32, tag="t1")
                t2 = work.tile([128, SUB], F32, tag="t2")
                nc.vector.stream_shuffle(qsh[:, :n], ps_in, swap_mask)
                nc.any.tensor_tensor(
                    t1[:, :n], ps_in, ropeC_sb[:, s0:s0 + n], AluOpType.mult)
                nc.any.tensor_tensor(
                    t2[:, :n], qsh[:, :n], ropeS_sb[:, s0:s0 + n],
                    AluOpType.mult)
                nc.any.tensor_tensor(out_ap, t1[:, :n], t2[:, :n],
                                     AluOpType.add)

            for sc in range(NSUB):
                b = sc // (NSUB // B)
                s0 = (sc % (NSUB // B)) * SUB       # position within batch
                half = sc % 2
                i_q = (sc % (NSUB // B)) // 2       # query chunk within batch

                xt = xpool.tile([128, KB * SUB], dt, tag="xt")
                nc.sync.dma_start(
                    xt.rearrange("p (kb t) -> p kb t", kb=KB),
                    xT.ap()[:, sc * SUB:(sc + 1) * SUB]
                      .rearrange("(kb p) t -> p kb t", p=128))

                # ---- q/k projections + rope, per head
                for h in range(HPC):
                    if half == 0:
                        qT_tiles[h] = qpool.tile([128, QCH], dt, tag="qT", name="qT")
                    if (b, h, i_q) not in kT_tiles:
                        kT_tiles[(b, h, i_q)] = kvpool.tile(
                            [128, QCH], dt, tag="kT", name="kT")
                    for (w_sb, dst) in ((wq_sb, qT_tiles[h]),
                                        (wk_sb, kT_tiles[(b, h, i_q)])):
                        psq = ps_proj.tile([128, SUB], F32, tag="proj")
                        for kb in range(KB):
                            nc.tensor.matmul(
                                psq[:],
                                _mm_view(w_sb[:, kb * HPC * DK + h * DK:
                                              kb * HPC * DK + (h + 1) * DK]),
                                _mm_view(xt[:, kb * SUB:(kb + 1) * SUB]),
                                start=(kb == 0), stop=(kb == KB - 1))
                        rope_combine(psq[:], dst[:, half * SUB:(half + 1) * SUB],
                                     s0, SUB)

                # ---- v projection (natural layout, both heads at once)
                for tb in range(SUB // 128):
                    jb_global = sc * 2 + tb          # 128-token block index
                    jb_b = jb_global - b * (S // 128)  # within batch
                    psv = ps_proj.tile([128, HPC * DK], F32, tag="proj")
                    for kb in range(KB):
                        nc.tensor.matmul(
                            psv[:],
                            _mm_view(xt[:, kb * SUB + tb * 128:
                                        kb * SUB + (tb + 1) * 128]),
                            _mm_view(wv_sb[:, kb * HPC * DK:
                                           (kb + 1) * HPC * DK]),
                            start=(kb == 0), stop=(kb == KB - 1))
                    vt = vpool.tile([128, HPC * DK], dt, tag="v")
                    nc.vector.tensor_copy(vt[:], psv[:])
                    v_tiles[(b, jb_b)] = vt

                # ---- attention for the completed query chunk
                if half != 1:
                    continue
                n_j = 4 * i_q + 4
                for h in range(HPC):
                    qT = qT_tiles[h]
                    ps_o = ps_out.tile([128, QCH], F32, tag="att_out")
                    ps_l = ps_misc.tile([1, QCH], F32, tag="l")
                    for j in range(n_j):
                        jc, jr = j // 4, j % 4
                        ps_s = ps_st.tile([JB, QCH], F32, tag="st")
                        nc.tensor.matmul(
                            ps_s[:],
                            _mm_view(kT_tiles[(b, h, jc)][:, jr * 128:
                                                          (jr + 1) * 128]),
                            _mm_view(qT[:]),
                            start=True, stop=True)
                        p_t = ppool.tile([JB, QCH], dt, tag="p")
                        if j >= 4 * i_q:           # diagonal block: mask
                            m = j - 4 * i_q
                            sm = work.tile([JB, QCH], F32, tag="sm")
                            nc.any.tensor_tensor(
                                sm[:], ps_s[:],
                                maskd_sb[:, m * QCH:(m + 1) * QCH],
                                AluOpType.add)
                            nc.scalar.activation(
                                p_t[:], sm[:], mybir.ActivationFunctionType.Exp)
                        else:
                            nc.scalar.activation(
                                p_t[:], ps_s[:],
                                mybir.ActivationFunctionType.Exp)
                        nc.tensor.matmul(
                            ps_l[:], _mm_view(ones_col[:]), _mm_view(p_t[:]),
                            start=(j == 0), stop=(j == n_j - 1))
                        nc.tensor.matmul(
                            ps_o[:],
                            _mm_view(v_tiles[(b, j)][:, h * DK:(h + 1) * DK]),
                            _mm_view(p_t[:]),
                            start=(j == 0), stop=(j == n_j - 1))
                    # normalize: y = outT / l  (broadcast 1/l over partitions)
                    r_sb = work.tile([1, QCH], F32, tag="r")
                    nc.vector.reciprocal(r_sb[:], ps_l[:])
                    ps_r = ps_misc.tile([128, QCH], F32, tag="R")
                    nc.tensor.matmul(
                        ps_r[:],
                        ones_row[:] if DT_MODE != "f32r" else _mm_view(ones_row[:]),
                        r_sb[:] if DT_MODE != "f32r" else _mm_view(r_sb[:]),
                        start=True, stop=True)
                    r_bc = work.tile([128, QCH], F32, tag="rbc")
                    nc.any.tensor_copy(r_bc[:], ps_r[:])
                    y_sb = work.tile([128, QCH], dt, tag="y")
                    nc.any.tensor_tensor(y_sb[:], ps_o[:], r_bc[:],
                                         AluOpType.mult)
                    jc_glob = b * 4 + i_q
                    nc.sync.dma_start(
                        y_a2a[jc_glob * HPC * DK + h * DK:
                              jc_glob * HPC * DK + (h + 1) * DK, :],
                        y_sb[:])

            # release the main-phase pools before the Wo phase opens
            st_main.close()

            # ---- AllToAll: head-shard -> token-shard
            nc.gpsimd.collective_compute(
                "AllToAll", AluOpType.bypass,
                replica_groups=[list(range(N_CORES))],
                ins=[y_a2a.opt()], outs=[yfull.opt()])

            # ---- output projection for this core's 512 tokens
            with tc.tile_pool(name="wo", bufs=1) as wop, \
                 tc.tile_pool(name="wout", bufs=4) as wout, \
                 tc.tile_pool(name="ps_wo", bufs=4, space="PSUM") as ps_wo:
                wo_sb = wop.tile([128, KB * D], dt)
                nc.sync.dma_start(
                    wo_sb.rearrange("p (kb e) -> p kb e", kb=KB),
                    woT.ap().rearrange("(kb p) e -> p kb e", p=128))
                yT_sb = wop.tile([128, KB * QCH], dt)
                nc.sync.dma_start(
                    yT_sb.rearrange("p (kb t) -> p kb t", kb=KB),
                    yfull.rearrange("(kb p) t -> p kb t", p=128))
                for eb in range(KB):
                    ps_w = ps_wo.tile([128, QCH], F32, tag="wo")
                    for db in range(KB):
                        nc.tensor.matmul(
                            ps_w[:],
                            _mm_view(wo_sb[:, db * D + eb * 128:
                                           db * D + (eb + 1) * 128]),
                            _mm_view(yT_sb[:, db * QCH:(db + 1) * QCH]),
                            start=(db == 0), stop=(db == KB - 1))
                    o_sb = wout.tile([128, QCH], F32, tag="osb")
                    nc.vector.tensor_copy(o_sb[:], ps_w[:])
                    nc.sync.dma_start(outT[eb * 128:(eb + 1) * 128, :], o_sb[:])

    nc.finalize()
    return nc


# ---------------------------------------------------------------- host
def _host_inputs(x, W_q, W_k, W_v, W_o):
    np_dt = _np_dt()
    xT = np.ascontiguousarray(
        x.reshape(TOK, D).T).astype(np_dt)                     # [D, TOK]
    woT = np.ascontiguousarray(W_o.T).astype(np_dt)            # [d, e]

    # RoPE tables, expanded to [DK, S] with interleaved pairs; the sign
    # table carries -sin on even rows, +sin on odd rows.
    i = np.arange(0, DK, 2, dtype=np.float32)
    theta = 1.0 / (ROPE_BASE ** (i / DK))                      # [64]
    pos = np.arange(S, dtype=np.float32)
    freqs = pos[:, None] * theta[None, :]                      # [S, 64]
    cos_t, sin_t = np.cos(freqs), np.sin(freqs)
    ropeC = np.empty((DK, S), np.float32)
    ropeS = np.empty((DK, S), np.float32)
    ropeC[0::2] = cos_t.T
    ropeC[1::2] = cos_t.T
    ropeS[0::2] = -sin_t.T
    ropeS[1::2] = sin_t.T

    # diagonal causal masks: block m (of the 4 key blocks overlapping a
    # 512-query chunk) keeps kk <= qq - 128*m
    kk = np.arange(JB)[:, None]
    qq = np.arange(QCH)[None, :]
    maskd = np.concatenate(
        [np.where(kk <= qq - 128 * m, 0.0, MASK_NEG).astype(np.float32)
         for m in range(4)], axis=1)                           # [128, 4*512]

    scale = 1.0 / np.sqrt(np.float32(DK))
    in_maps = []
    for c in range(N_CORES):
        rows = slice(c * HPC * DK, (c + 1) * HPC * DK)
        in_maps.append({
            "xT": xT,
            "wqT": np.ascontiguousarray((W_q[rows] * scale).T).astype(np_dt),
            "wkT": np.ascontiguousarray(W_k[rows].T).astype(np_dt),
            "wvT": np.ascontiguousarray(W_v[rows].T).astype(np_dt),
            "woT": woT,
            "ropeC": ropeC,
            "ropeS": ropeS,
            "maskd": maskd,
        })
    return in_maps


def kernel(x, W_q, W_k, W_v, W_o):
    x = np.asarray(x, dtype=np.float32)
    W_q = np.asarray(W_q, dtype=np.float32)
    W_k = np.asarray(W_k, dtype=np.float32)
    W_v = np.asarray(W_v, dtype=np.float32)
    W_o = np.asarray(W_o, dtype=np.float32)

    if "nc" not in _CACHE:
        _CACHE["nc"] = _build_nc()
    nc = _CACHE["nc"]

    in_maps = _host_inputs(x, W_q, W_k, W_v, W_o)
    res = bass_utils.run_bass_kernel_spmd(
        nc, in_maps, core_ids=list(range(N_CORES)))

    # outT per core: [D, 512] fp32 for tokens [c*512:(c+1)*512]
    out_T = np.concatenate([res.results[c]["outT"] for c in range(N_CORES)],
                           axis=1)                             # [D, TOK]
    return np.ascontiguousarray(out_T.T).reshape(B, S, D).astype(np.float32)
